# revision 1
# baseline (speedup 1.0000x reference)
"""Trainium2 Bass kernel for nn_MultiHeadAttention_84791244358011.

Linear (ELU feature-map) attention:
    x_norm = LayerNorm(x)                      # eps=1e-12
    q = x_norm @ Wq.T + bq ; k,v = x @ W.T + b # per-head [S, 64]
    eq/ek = l2norm(elu(q/k)) per token over head_dim
    kv = ek^T @ v per head [64, 64]; ctx = eq @ kv / 8
    out = ctx @ Wo.T + bo + x

Sharding: data-parallel over batch B=8 — one batch element per NeuronCore,
no collectives. Weights are pre-transposed host-side (static prep):
    wqt[i,j] = Wq[j,i]*gamma[i]; wkt/wvt = W.T; wot = Wo.T*(1/sqrt(64))
    bq_eff = bq + beta @ Wq.T
so every on-device matmul contracts over the SBUF partition dim. Matmuls
run in float32r (full PE rate, ~2^-13 operand rounding).

Per core, two passes over the 32 token-tiles (128 tokens each):
  pass A: load x tile -> LN stats -> z=(x-mu)*rstd; PE-transpose x and z;
          k/v projections from x^T (biases folded in as K=1 rank-1
          matmuls); elu+l2norm on k; accumulate per-head kv state in
          PSUM; spill z^T to DRAM scratch.
  pass B: reload z^T -> q projection -> elu+l2norm -> PE-transpose eq;
          per 512-token chunk ctx^T = kv @ eq^T (row-tiled 64x64 pairs);
          out = ctx^T.T @ wot (+bo via rank-1) + x.

Scheduling notes (from CoreSim engine-occupancy analysis):
  - all rsqrt/sqrt via Newton on DVE (quake seed + 2 iters) so the scalar
    engine stays on one ACT table set (exp_and_others) — table loads are
    1.28us each;
  - k/v/q/o PSUM tiles are bank-sized halves with bufs=2 so the PE runs
    ahead while the DVE drains the previous tile's PSUM;
  - DMA load is spread across the SP/ACT HWDGE queues and the gpsimd
    SWDGE queue;
  - PSUM start=True pends-zero the whole 2KB bank: only the first matmul
    per bank may set it.
"""

import numpy as np

import concourse.bass as bass
import concourse.mybir as mybir
import concourse.tile as tile
from concourse import bacc
from concourse.masks import make_identity

B, S, HID = 8, 4096, 1024
NH, HD = 16, 64
P = 128
NT = S // P            # 32 token tiles
NC = HID // P          # 8 feature chunks
HNH = NH // 2          # heads per psum half
CHUNK = 4              # token tiles per ctx chunk (512 tokens)
NCHUNKS = NT // CHUNK
LN_EPS = 1e-12

F32 = mybir.dt.float32
F32R = mybir.dt.float32r
I32 = mybir.dt.int32
AF = mybir.ActivationFunctionType
OP = mybir.AluOpType


def _rsqrt(nc, pool, consts, src, n, name):
    """1/sqrt(src[:, :n]) on DVE: quake-III seed + 2 Newton iterations."""
    magic_b, one_b = consts
    e = nc.vector
    shi = pool.tile([P, n], I32, tag=f"rq_sh{n}", bufs=4, name=f"{name}_shi")
    e.tensor_tensor(shi, src.bitcast(I32), one_b[:, 0:1].to_broadcast((P, n)),
                    OP.arith_shift_right)
    y0i = pool.tile([P, n], I32, tag=f"rq_y0{n}", bufs=4, name=f"{name}_y0i")
    e.tensor_tensor(y0i, magic_b[:, 0:1].to_broadcast((P, n)), shi, OP.subtract)
    h = pool.tile([P, n], F32, tag=f"rq_h{n}", bufs=4, name=f"{name}_h")
    e.tensor_scalar(h, src, -0.5, None, OP.mult)
    cur = y0i[:].bitcast(F32)
    for it in range(2):
        t = pool.tile([P, n], F32, tag=f"rq_t{n}_{it}", bufs=4,
                      name=f"{name}_t{it}")
        e.tensor_tensor(t, cur, cur, OP.mult)
        e.tensor_tensor(t, t, h, OP.mult)
        e.tensor_scalar(t, t, 1.5, None, OP.add)
        y = pool.tile([P, n], F32, tag=f"rq_y{n}_{it}", bufs=4,
                      name=f"{name}_y{it}")
        e.tensor_tensor(y, cur, t, OP.mult)
        cur = y
    return cur


def _elu_half(nc, pool, ps_half, bias_rep, half, raw, name):
    """raw[:, half-heads] = elu(ps_half + bias) = (max(x,0)-1) + exp(min(x,0))."""
    sl = slice(half * 512, (half + 1) * 512)
    xb = pool.tile([P, HID // 2], F32, tag="eb", bufs=3, name=f"{name}_xb")
    nc.vector.tensor_tensor(xb, ps_half, bias_rep[:, sl], OP.add)
    r = pool.tile([P, HID // 2], F32, tag="er", bufs=3, name=f"{name}_r")
    nc.scalar.activation(r, xb, AF.Relu, scale=-1.0)
    e = pool.tile([P, HID // 2], F32, tag="ee", bufs=3, name=f"{name}_e")
    nc.scalar.activation(e, r, AF.Exp, scale=-1.0)
    m = pool.tile([P, HID // 2], F32, tag="em", bufs=3, name=f"{name}_m")
    nc.vector.tensor_scalar(m, xb, 0.0, 1.0, OP.max, OP.subtract)
    nc.gpsimd.tensor_tensor(
        raw[:, half * HNH:(half + 1) * HNH, :].rearrange("p h d -> p (h d)"),
        m, e, OP.add)


def build_nc(debug=False, loop_n=1):
    nc = bacc.Bacc("TRN2", target_bir_lowering=False, enable_partition_id=False)
    dbg = {}
    if debug:
        dbg["ek0"] = nc.dram_tensor("dbg_ek0", [P, HID], F32, kind="ExternalOutput")
        dbg["v0"] = nc.dram_tensor("dbg_v0", [P, HID], F32, kind="ExternalOutput")
        dbg["kv"] = nc.dram_tensor("dbg_kv", [HD, NH * HD], F32,
                                   kind="ExternalOutput")
        dbg["eq0"] = nc.dram_tensor("dbg_eq0", [P, HID], F32, kind="ExternalOutput")

    x_d = nc.dram_tensor("x", [S, HID], F32, kind="ExternalInput")
    wqt_d = nc.dram_tensor("wqt", [HID, HID], F32, kind="ExternalInput")
    wkt_d = nc.dram_tensor("wkt", [HID, HID], F32, kind="ExternalInput")
    wvt_d = nc.dram_tensor("wvt", [HID, HID], F32, kind="ExternalInput")
    wot_d = nc.dram_tensor("wot", [HID, HID], F32, kind="ExternalInput")
    bq_d = nc.dram_tensor("bq", [1, HID], F32, kind="ExternalInput")
    bk_d = nc.dram_tensor("bk", [1, HID], F32, kind="ExternalInput")
    bv_d = nc.dram_tensor("bv", [1, HID], F32, kind="ExternalInput")
    bo_d = nc.dram_tensor("bo", [1, HID], F32, kind="ExternalInput")
    out_d = nc.dram_tensor("out", [S, HID], F32, kind="ExternalOutput")
    zt_d = nc.dram_tensor("zt_spill", [NT, P, HID], F32)

    import contextlib

    with tile.TileContext(nc) as tc, contextlib.ExitStack() as ctx:
        persist = ctx.enter_context(tc.tile_pool(name="persist", bufs=1))
        wpool = ctx.enter_context(tc.tile_pool(name="weights", bufs=1))

        ident = persist.tile([P, P], F32)
        make_identity(nc, ident)
        ident_r = persist.tile([P, P], F32R)
        nc.scalar.activation(ident_r, ident, AF.Copy)
        magic_t = persist.tile([P, 1], I32)
        nc.gpsimd.memset(magic_t, 0x5F3759DF)
        one_t = persist.tile([P, 1], I32)
        nc.gpsimd.memset(one_t, 1)
        consts = (magic_t, one_t)
        kv_sb = persist.tile([P, HNH * HD], F32R)   # packed kv state
        brep = {}
        for nm, d in (("bq", bq_d), ("bk", bk_d), ("bv", bv_d), ("bo", bo_d)):
            t_ = persist.tile([P, HID], F32, name=f"{nm}_rep")
            h = d.ap()
            nc.gpsimd.dma_start(
                t_, bass.AP(tensor=h.tensor, offset=h.offset,
                            ap=[[0, P], [1, HID]]))
            brep[nm] = t_

        def proj_half(ps, lhs_fn, w_sb, half):
            """ps[P,512] = sum_c lhs(c).T @ w[c, half]."""
            sl = slice(half * 512, (half + 1) * 512)
            for c in range(NC):
                nc.tensor.matmul(ps, lhs_fn(c), w_sb[:, c, sl],
                                 start=(c == 0), stop=(c == NC - 1))

        _loop = tc.For_i(0, loop_n, 1) if loop_n > 1 else contextlib.nullcontext(0)
        with _loop:
            # ------------- pass A: k/v projections + kv state -------------
            wk_sb = wpool.tile([P, NC, HID], F32R, tag="wA", name="wk_sb")
            nc.sync.dma_start(
                wk_sb, wkt_d.ap().rearrange("(c p) j -> p c j", p=P).bitcast(F32R))
            wv_sb = wpool.tile([P, NC, HID], F32R, tag="wB", name="wv_sb")
            nc.sync.dma_start(
                wv_sb, wvt_d.ap().rearrange("(c p) j -> p c j", p=P).bitcast(F32R))

            with tc.tile_pool(name="sbufA", bufs=1) as sa, \
                 tc.tile_pool(name="psumA", bufs=1, space="PSUM") as pa:
                # kv state: [64, NH*64] — every head at partition base 0
                kv_ps = pa.tile([HD, NH * HD], F32, tag="kv", name="kv_ps")

                for t in range(NT):
                    x_t = sa.tile([P, HID], F32, tag="x", bufs=4, name=f"x_{t}")
                    nc.scalar.dma_start(x_t, x_d.ap()[t * P:(t + 1) * P, :])

                    # LayerNorm stats
                    stats = sa.tile([P, 2, 6], F32, tag="st", bufs=4,
                                    name=f"st_{t}")
                    xg = x_t[:].rearrange("p (g d) -> p g d", g=2)
                    for g in range(2):
                        nc.vector.bn_stats(stats[:, g, :], xg[:, g, :])
                    mv = sa.tile([P, 2], F32, tag="mv", bufs=4, name=f"mv_{t}")
                    nc.vector.bn_aggr(mv, stats)
                    vpe = sa.tile([P, 1], F32, tag="sd", bufs=4, name=f"sd_{t}")
                    nc.vector.tensor_scalar(vpe, mv[:, 1:2], LN_EPS, None, OP.add)
                    rstd = _rsqrt(nc, sa, consts, vpe[:], 1, f"rs_{t}")
                    z_t = sa.tile([P, HID], F32, tag="z", bufs=3, name=f"z_{t}")
                    nc.vector.tensor_scalar(
                        z_t, x_t, mv[:, 0:1], rstd, OP.subtract, OP.mult)

                    # transpose x and z (PE), copy out, spill z^T
                    xT = sa.tile([P, NC, P], F32R, tag="xT", bufs=3,
                                 name=f"xT_{t}")
                    zT = sa.tile([P, NC, P], F32, tag="zT", bufs=2,
                                 name=f"zT_{t}")
                    for src, dst in ((x_t, xT), (z_t, zT)):
                        for half in range(2):
                            tp = pa.tile([P, 4 * P], F32, tag="tp", bufs=2,
                                         name=f"tp_{t}_{half}")
                            for b_ in range(4):
                                blk = half * 4 + b_
                                nc.tensor.transpose(
                                    tp[:, b_ * P:(b_ + 1) * P],
                                    src[:, blk * P:(blk + 1) * P], ident)
                            nc.vector.tensor_copy(
                                dst[:].rearrange("p c s -> p (c s)")[
                                    :, half * 4 * P:(half + 1) * 4 * P], tp)
                    nc.sync.dma_start(zt_d.ap()[t],
                                      zT[:].rearrange("p c s -> p (c s)"))

                    def xT_lhs(c, xT=xT):
                        return xT[:, c, :]

                    # k/v projections into half-bank psums; elu on k halves
                    raw = sa.tile([P, NH, HD], F32, tag="eraw", bufs=2,
                                  name=f"ekraw{t}")
                    v_sb = sa.tile([P, NH, HD], F32R, tag="vsb", bufs=2,
                                   name=f"v_sb{t}")
                    for half in range(2):
                        k_ps = pa.tile([P, 512], F32, tag="kh", bufs=2,
                                       name=f"k_ps{t}_{half}")
                        proj_half(k_ps, xT_lhs, wk_sb, half)
                        _elu_half(nc, sa, k_ps, brep["bk"], half, raw,
                                  f"ek{t}_{half}")
                        v_ps = pa.tile([P, 512], F32, tag="vh", bufs=2,
                                       name=f"v_ps{t}_{half}")
                        proj_half(v_ps, xT_lhs, wv_sb, half)
                        nc.vector.tensor_tensor(
                            v_sb[:, half * HNH:(half + 1) * HNH, :].rearrange(
                                "p h d -> p (h d)"), v_ps,
                            brep["bv"][:, half * 512:(half + 1) * 512], OP.add)

                    # per-head l2 norm of elu(k): sumsq -> rsqrt -> scale
                    sq = sa.tile([P, NH, HD], F32, tag="esq", bufs=2,
                                 name=f"sq{t}")
                    nc.scalar.activation(
                        sq[:].rearrange("p h d -> p (h d)"),
                        raw[:].rearrange("p h d -> p (h d)"), AF.Square)
                    ss = sa.tile([P, NH], F32, tag="ess", bufs=4, name=f"ss{t}")
                    nc.vector.tensor_reduce(ss, sq, mybir.AxisListType.X, OP.add)
                    rn = _rsqrt(nc, sa, consts, ss[:], NH, f"ekrn{t}")
                    ek = sa.tile([P, NH, HD], F32R, tag="eout", bufs=2,
                                 name=f"ek{t}")
                    nc.vector.tensor_tensor(
                        ek, raw, rn[:, :, None].to_broadcast((P, NH, HD)),
                        OP.mult)
                    if debug and t == 0:
                        nc.sync.dma_start(
                            dbg["ek0"].ap(),
                            ek[:].rearrange("p h d -> p (h d)").bitcast(F32))
                        nc.sync.dma_start(
                            dbg["v0"].ap(),
                            v_sb[:].rearrange("p h d -> p (h d)").bitcast(F32))

                    # kv state accumulation (start only on first mm per bank)
                    for h in range(NH):
                        nc.tensor.matmul(
                            kv_ps[:, h * HD:(h + 1) * HD],
                            ek[:, h, :], v_sb[:, h, :],
                            start=(t == 0 and h % 8 == 0), stop=(t == NT - 1),
                            skip_group_check=True)

                # kv state -> SBUF (f32r), packed 2 heads per 128 partitions
                kvv = kv_ps[:].rearrange("p (a r d) -> p a r d", r=2, d=HD)
                kvb = kv_sb[:].rearrange("p (a d) -> p a d", d=HD)
                nc.vector.tensor_copy(kvb[0:HD], kvv[:, :, 0, :])
                nc.vector.tensor_copy(kvb[HD:P], kvv[:, :, 1, :])
                if debug:
                    kvstage = sa.tile([HD, NH * HD], F32, name="kvstage")
                    nc.vector.tensor_copy(kvstage, kv_ps)
                    nc.sync.dma_start(dbg["kv"].ap(), kvstage)

            # ------------- pass B: q -> ctx -> out ------------------------
            wq_sb = wpool.tile([P, NC, HID], F32R, tag="wA", name="wq_sb")
            nc.sync.dma_start(
                wq_sb, wqt_d.ap().rearrange("(c p) j -> p c j", p=P).bitcast(F32R))
            wo_sb = wpool.tile([P, NC, HID], F32R, tag="wB", name="wo_sb")
            nc.sync.dma_start(
                wo_sb, wot_d.ap().rearrange("(c p) j -> p c j", p=P).bitcast(F32R))

            with tc.tile_pool(name="sbufB", bufs=1) as sb, \
                 tc.tile_pool(name="psumB", bufs=1, space="PSUM") as pb:
                for ch in range(NCHUNKS):
                    eqT = sb.tile([P, NC, CHUNK * P], F32R, tag="eqT", bufs=1,
                                  name=f"eqT{ch}")
                    for tl in range(CHUNK):
                        t = ch * CHUNK + tl
                        zt_sb = sb.tile([P, HID], F32R, tag="zt", bufs=2,
                                        name=f"zt{t}")
                        nc.scalar.dma_start(zt_sb, zt_d.ap()[t].bitcast(F32R))

                        def zt_lhs(c, zt_sb=zt_sb):
                            return zt_sb[:, c * P:(c + 1) * P]

                        raw = sb.tile([P, NH, HD], F32, tag="eraw", bufs=2,
                                      name=f"eqraw{t}")
                        for half in range(2):
                            q_ps = pb.tile([P, 512], F32, tag="qh", bufs=2,
                                           name=f"q_ps{t}_{half}")
                            proj_half(q_ps, zt_lhs, wq_sb, half)
                            _elu_half(nc, sb, q_ps, brep["bq"], half, raw,
                                      f"eq{t}_{half}")

                        sq = sb.tile([P, NH, HD], F32, tag="esq", bufs=2,
                                     name=f"sqB{t}")
                        nc.scalar.activation(
                            sq[:].rearrange("p h d -> p (h d)"),
                            raw[:].rearrange("p h d -> p (h d)"), AF.Square)
                        ss = sb.tile([P, NH], F32, tag="ess", bufs=4,
                                     name=f"ssB{t}")
                        nc.vector.tensor_reduce(ss, sq, mybir.AxisListType.X,
                                                OP.add)
                        rn = _rsqrt(nc, sb, consts, ss[:], NH, f"eqrn{t}")
                        eq = sb.tile([P, NH, HD], F32R, tag="eout", bufs=2,
                                     name=f"eq{t}")
                        nc.vector.tensor_tensor(
                            eq, raw, rn[:, :, None].to_broadcast((P, NH, HD)),
                            OP.mult)
                        eqf = eq[:].rearrange("p h d -> p (h d)")
                        if debug and t == 0:
                            nc.sync.dma_start(dbg["eq0"].ap(), eqf.bitcast(F32))

                        for half in range(2):
                            tp = pb.tile([P, 4 * P], F32, tag="tp", bufs=2,
                                         name=f"tpB_{t}_{half}")
                            for b_ in range(4):
                                blk = half * 4 + b_
                                nc.tensor.transpose(
                                    tp[:, b_ * P:(b_ + 1) * P].bitcast(F32R),
                                    eqf[:, blk * P:(blk + 1) * P], ident_r)
                            nc.vector.tensor_copy(
                                eqT[:, half * 4:(half + 1) * 4,
                                    tl * P:(tl + 1) * P], tp)

                    # ctx^T for this chunk: per j-tile two row-tiled 64-K mms
                    ctxT = sb.tile([P, NC, CHUNK * P], F32R, tag="ctxT", bufs=1,
                                   name=f"ctxT{ch}")
                    for jt in range(NC):
                        c_pse = pb.tile([HD, CHUNK * P], F32, tag="ctxe",
                                        bufs=1, name=f"c_pse{ch}_{jt}")
                        c_pso = pb.tile([HD, CHUNK * P], F32, tag="ctxo",
                                        bufs=1, name=f"c_pso{ch}_{jt}")
                        nc.tensor.matmul(
                            c_pse, kv_sb[0:HD, jt * HD:(jt + 1) * HD],
                            eqT[0:HD, jt, :], start=True, stop=True)
                        nc.tensor.matmul(
                            c_pso, kv_sb[HD:P, jt * HD:(jt + 1) * HD],
                            eqT[HD:P, jt, :], start=True, stop=True)
                        nc.scalar.copy(ctxT[0:HD, jt, :], c_pse)
                        nc.scalar.copy(ctxT[HD:P, jt, :], c_pso)

                    for tl in range(CHUNK):
                        t = ch * CHUNK + tl

                        def ctx_lhs(c, ctxT=ctxT, tl=tl):
                            return ctxT[:, c, tl * P:(tl + 1) * P]

                        x_t2 = sb.tile([P, HID], F32, tag="x2", bufs=2,
                                       name=f"x2_{t}")
                        nc.sync.dma_start(x_t2, x_d.ap()[t * P:(t + 1) * P, :])
                        xb2 = sb.tile([P, HID], F32, tag="xb2", bufs=2,
                                      name=f"xb2_{t}")
                        nc.gpsimd.tensor_tensor(xb2, x_t2, brep["bo"], OP.add)
                        out_sb = sb.tile([P, HID], F32, tag="osb", bufs=2,
                                         name=f"out_{t}")
                        for half in range(2):
                            o_ps = pb.tile([P, 512], F32, tag="oh", bufs=2,
                                           name=f"o_ps{t}_{half}")
                            proj_half(o_ps, ctx_lhs, wo_sb, half)
                            sl = slice(half * 512, (half + 1) * 512)
                            nc.vector.tensor_tensor(
                                out_sb[:, sl], o_ps, xb2[:, sl], OP.add)
                        nc.gpsimd.dma_start(
                            out_d.ap()[t * P:(t + 1) * P, :], out_sb)

    nc.compile()
    return nc


_RUNNER = {}
_NC_CACHE = None


def _get_runner(loop_n=1):
    global _NC_CACHE
    if loop_n in _RUNNER:
        return _RUNNER[loop_n]

    import jax
    from jax.sharding import Mesh, PartitionSpec
    from jax.experimental.shard_map import shard_map
    from concourse.bass2jax import _bass_exec_p, install_neuronx_cc_hook

    install_neuronx_cc_hook()
    nc = build_nc(loop_n=loop_n)
    if loop_n == 1:
        _NC_CACHE = nc

    in_names = []
    out_names = []
    out_avals = []
    for alloc in nc.m.functions[0].allocations:
        if not isinstance(alloc, mybir.MemoryLocationSet):
            continue
        name = alloc.memorylocations[0].name
        if alloc.kind == "ExternalInput":
            in_names.append(name)
        elif alloc.kind == "ExternalOutput":
            out_names.append(name)
            out_avals.append(
                jax.core.ShapedArray(tuple(alloc.tensor_shape),
                                     mybir.dt.np(alloc.dtype)))
    n_params = len(in_names)
    all_in_names = in_names + out_names

    def _body(*args):
        outs = _bass_exec_p.bind(
            *args,
            out_avals=tuple(out_avals),
            in_names=tuple(all_in_names),
            out_names=tuple(out_names),
            lowering_input_output_aliases=(),
            sim_require_finite=True,
            sim_require_nnan=True,
            nc=nc,
        )
        return tuple(outs)

    devices = jax.devices()[:B]
    mesh = Mesh(np.asarray(devices), ("core",))
    n_outs = len(out_names)
    fn = jax.jit(
        shard_map(
            _body, mesh=mesh,
            in_specs=(PartitionSpec("core"),) * (n_params + n_outs),
            out_specs=(PartitionSpec("core"),) * n_outs,
            check_rep=False,
        ),
        keep_unused=True,
    )
    _RUNNER[loop_n] = (fn, in_names, out_names, out_avals)
    return _RUNNER[loop_n]


def prep_inputs(input_tensor, attention_mask, ln_gamma, ln_beta,
                Wq, bq, Wk, bk, Wv, bv, Wo, bo):
    """Host-side static prep: transpose weights, fold gamma/beta/scale."""
    f = np.float32
    x = np.ascontiguousarray(np.asarray(input_tensor, f))
    g = np.asarray(ln_gamma, f)
    be = np.asarray(ln_beta, f)
    Wq = np.asarray(Wq, f); Wk = np.asarray(Wk, f)
    Wv = np.asarray(Wv, f); Wo = np.asarray(Wo, f)
    wqt = np.ascontiguousarray((Wq * g[None, :]).T)        # [i, j]
    wkt = np.ascontiguousarray(Wk.T)
    wvt = np.ascontiguousarray(Wv.T)
    wot = np.ascontiguousarray(Wo.T * np.float32(1.0 / np.sqrt(HD)))
    bq_eff = (np.asarray(bq, f) + be @ Wq.T).astype(f)
    per_core = {
        "wqt": wqt, "wkt": wkt, "wvt": wvt, "wot": wot,
        "bq": bq_eff.reshape(1, HID),
        "bk": np.asarray(bk, f).reshape(1, HID),
        "bv": np.asarray(bv, f).reshape(1, HID),
        "bo": np.asarray(bo, f).reshape(1, HID),
    }
    return x, per_core


def kernel(**inputs) -> np.ndarray:
    x, per_core = prep_inputs(**inputs)
    fn, in_names, out_names, out_avals = _get_runner()

    concat_in = []
    for name in in_names:
        if name == "x":
            concat_in.append(x.reshape(B * S, HID))
        else:
            concat_in.append(np.concatenate([per_core[name]] * B, axis=0))
    concat_zeros = [
        np.zeros((B * av.shape[0], *av.shape[1:]), av.dtype) for av in out_avals
    ]
    out_arrs = fn(*concat_in, *concat_zeros)
    out = np.asarray(out_arrs[out_names.index("out")])
    return out.reshape(B, S, HID)



# revision 3
# speedup vs baseline: 1.1625x; 1.1625x over previous
"""Trainium2 Bass kernel for nn_MultiHeadAttention_84791244358011.

Linear (ELU feature-map) attention:
    x_norm = LayerNorm(x)                      # eps=1e-12
    q = x_norm @ Wq.T + bq ; k,v = x @ W.T + b # per-head [S, 64]
    eq/ek = l2norm(elu(q/k)) per token over head_dim
    kv = ek^T @ v per head [64, 64]; ctx = eq @ kv / 8
    out = ctx @ Wo.T + bo + x

Sharding: data-parallel over batch B=8 — one batch element per NeuronCore,
no collectives.

v2 design (single pass, bf16 dataflow):
  - x converted to bf16 host-side (halves DMA; LN stats in fp32).
  - Weights pre-transposed + bf16 host-side:
        wqt[i,j] = Wq[j,i]*gamma[i]; wkt/wvt = W.T; wot = Wo.T/sqrt(64)
    every matmul contracts over the SBUF partition dim at 1 cycle/row.
  - LayerNorm folded into the q projection:
        q = rstd * (x @ wqt - mu * colsum(wqt))
    the -mu*colsum term is a rank-1 (K=1) matmul into the same PSUM
    accumulation; rstd rides the ACT `scale=` operand of the elu reads.
  - Single pass A per 128-token tile: transpose x; k/v/q projections;
    elu = Relu(ps) + (min(Exp(ps),1)-1); batched l2-norms with
    rsqrt = Exp(-0.5*Ln(ss)) on ACT (the act table pass is pinned to one
    table containing exp/ln/square/relu/copy — no table thrash);
    per-head-pair kv-state matmuls (8 of [128,128], diagonal blocks used);
    eq^T kept resident in SBUF (bf16) — no DRAM spill.
    PE work of tile t-1's tail (kv matmuls + eq^T transposes) is emitted
    after tile t's projections so the elu/norm chain of t-1 overlaps the
    PE-heavy front of t.
  - Pass B per 512-token chunk: ctx^T = kv @ eq^T; out = ctx^T.T @ wot + x.

Bias handling: when bq_eff (= bq + beta @ Wq.T), bk, bv, bo are all zero
(true for this problem's inputs) the bias adds are compiled out; a general
variant with the adds is built if any bias is nonzero.
"""

import functools

import numpy as np

import concourse.bass as bass
import concourse.mybir as mybir
import concourse.tile as tile
from concourse import bacc
from concourse.masks import make_identity

B, S, HID = 8, 4096, 1024
NH, HD = 16, 64
P = 128
NT = S // P            # 32 token tiles
NC = HID // P          # 8 feature chunks
CHUNK = 4              # token tiles per ctx chunk (512 tokens)
NCHUNKS = NT // CHUNK
LN_EPS = 1e-12

F32 = mybir.dt.float32
BF16 = mybir.dt.bfloat16
AF = mybir.ActivationFunctionType
OP = mybir.AluOpType

_ACT_PATCHED = False


def _patch_act_tables():
    """Pin the ACT table pass to one function set containing every func we
    use (exp/ln/square/relu/copy/identity), so it is loaded once instead of
    thrashing between the exp and ln sets. Set ids and contents are
    unchanged — other sets merely stop advertising our funcs."""
    global _ACT_PATCHED
    if _ACT_PATCHED:
        return
    import concourse.hw_specs as hws

    need = {AF.Exp, AF.Ln, AF.Square, AF.Relu, AF.Copy, AF.Identity}
    orig = hws.get_activation_tables

    @functools.cache
    def patched(arch):
        d = orig(arch)
        best = None
        for name, s in d.items():
            if need <= s:
                best = name
                break
        if best is None:
            return d
        return {name: (s if name == best else (s - need))
                for name, s in d.items()}

    bacc.get_activation_tables = patched
    hws.get_activation_tables = patched
    _ACT_PATCHED = True


def build_nc(loop_n=1, with_bias=False):
    _patch_act_tables()
    nc = bacc.Bacc("TRN2", target_bir_lowering=False, enable_partition_id=False)

    x_d = nc.dram_tensor("x", [S, HID], BF16, kind="ExternalInput")
    wqt_d = nc.dram_tensor("wqt", [HID, HID], BF16, kind="ExternalInput")
    wkt_d = nc.dram_tensor("wkt", [HID, HID], BF16, kind="ExternalInput")
    wvt_d = nc.dram_tensor("wvt", [HID, HID], BF16, kind="ExternalInput")
    wot_d = nc.dram_tensor("wot", [HID, HID], BF16, kind="ExternalInput")
    csq_d = nc.dram_tensor("csq", [1, HID], BF16, kind="ExternalInput")
    b_d = {}
    if with_bias:
        for nm in ("bq", "bk", "bv", "bo"):
            b_d[nm] = nc.dram_tensor(nm, [1, HID], F32, kind="ExternalInput")
    out_d = nc.dram_tensor("out", [S, HID], BF16, kind="ExternalOutput")

    import contextlib

    with tile.TileContext(nc) as tc, contextlib.ExitStack() as ctx:
        persist = ctx.enter_context(tc.tile_pool(name="persist", bufs=1))

        ident = persist.tile([P, P], BF16)
        make_identity(nc, ident)
        eqT = persist.tile([P, NC, S], BF16, name="eqT")      # 64KB/part
        kv_sb = persist.tile([P, (NH // 2) * HD], BF16, name="kv_sb")
        csq_sb = persist.tile([1, HID], BF16, name="csq_sb")
        nc.sync.dma_start(csq_sb, csq_d.ap())
        w_sb = {}
        for nm, d in (("wq", wqt_d), ("wk", wkt_d), ("wv", wvt_d),
                      ("wo", wot_d)):
            t_ = persist.tile([P, NC, HID], BF16, name=f"{nm}_sb")
            nc.sync.dma_start(t_, d.ap().rearrange("(c p) j -> p c j", p=P))
            w_sb[nm] = t_
        brep = {}
        if with_bias:
            for nm, d in b_d.items():
                t_ = persist.tile([P, HID], F32, name=f"{nm}_rep")
                h = d.ap()
                nc.gpsimd.dma_start(
                    t_, bass.AP(tensor=h.tensor, offset=h.offset,
                                ap=[[0, P], [1, HID]]))
                brep[nm] = t_

        _loop = tc.For_i(0, loop_n, 1) if loop_n > 1 else contextlib.nullcontext(0)
        with _loop:
            # ---------------- pass A ----------------
            with tc.tile_pool(name="sbufA", bufs=1) as sa, \
                 tc.tile_pool(name="psumA", bufs=1, space="PSUM") as pa:
                # kv state: head pairs a=0..7, [128, 128] block each; the
                # diagonal 64x64 blocks are the per-head kv states.
                kv_ps = pa.tile([P, 8 * P], F32, tag="kv", name="kv_ps")

                def tile_front(t):
                    """DMA + transposes + stats + projections + elu + norms.
                    Returns (ek, v_sb, eq) bf16 tiles for the tail."""
                    xt = sa.tile([P, HID], BF16, tag="x", bufs=4,
                                 name=f"x_{t}")
                    nc.scalar.dma_start(xt, x_d.ap()[t * P:(t + 1) * P, :])

                    xT = sa.tile([P, NC, P], BF16, tag="xT", bufs=3,
                                 name=f"xT_{t}")
                    for half in range(2):
                        tp = pa.tile([P, 4 * P], BF16, tag="tp", bufs=2,
                                     name=f"tp_{t}_{half}")
                        for b_ in range(4):
                            blk = half * 4 + b_
                            nc.tensor.transpose(
                                tp[:, b_ * P:(b_ + 1) * P],
                                xt[:, blk * P:(blk + 1) * P], ident)
                        nc.vector.tensor_copy(
                            xT[:].rearrange("p c s -> p (c s)")[
                                :, half * 4 * P:(half + 1) * 4 * P], tp)

                    # LayerNorm stats (fp32)
                    stats = sa.tile([P, 2, 6], F32, tag="st", bufs=4,
                                    name=f"st_{t}")
                    xg = xt[:].rearrange("p (g d) -> p g d", g=2)
                    for g in range(2):
                        nc.vector.bn_stats(stats[:, g, :], xg[:, g, :])
                    mv = sa.tile([P, 2], F32, tag="mv", bufs=4, name=f"mv_{t}")
                    nc.vector.bn_aggr(mv, stats)
                    vpe = sa.tile([P, 1], F32, tag="vpe", bufs=4,
                                  name=f"vpe_{t}")
                    nc.vector.tensor_scalar(vpe, mv[:, 1:2], LN_EPS, None,
                                            OP.add)
                    lnv = sa.tile([P, 1], F32, tag="lnv", bufs=4,
                                  name=f"lnv_{t}")
                    nc.scalar.activation(lnv, vpe, AF.Ln)
                    rstd = sa.tile([P, 1], F32, tag="rstd", bufs=4,
                                   name=f"rstd_{t}")
                    nc.scalar.activation(rstd, lnv, AF.Exp, scale=-0.5)
                    negmu = sa.tile([P, 1], BF16, tag="nmu", bufs=4,
                                    name=f"nmu_{t}")
                    nc.vector.tensor_scalar(negmu, mv[:, 0:1], -1.0, None,
                                            OP.mult)
                    tpn = pa.tile([P, 4 * P], BF16, tag="tp", bufs=2,
                                  name=f"tpn_{t}")
                    nc.tensor.transpose(tpn[0:1, 0:P], negmu, ident)
                    nmrow = sa.tile([1, P], BF16, tag="nmrow", bufs=3,
                                    name=f"nmrow_{t}")
                    nc.vector.tensor_copy(nmrow, tpn[0:1, 0:P])

                    # raw = [elu(k) | elu(q)] packed [P, 2048]
                    raw = sa.tile([P, 2 * HID], BF16, tag="raw", bufs=2,
                                  name=f"raw_{t}")
                    v_sb = sa.tile([P, NH, HD], BF16, tag="vsb", bufs=2,
                                   name=f"v_{t}")
                    vflat = v_sb[:].rearrange("p h d -> p (h d)")

                    def elu_into(dst, ps, scale, name):
                        # dst = Relu(ps*scale) + (min(Exp(ps*scale),1) - 1)
                        src = ps
                        if with_bias:
                            # general path: materialize ps*scale + bias first
                            bnm = "bq" if name.startswith("q") else "bk"
                            sl_ = slice(int(name.split("_")[1]) * 512,
                                        (int(name.split("_")[1]) + 1) * 512)
                            xb = sa.tile([P, 512], BF16, tag="xb", bufs=3,
                                         name=f"xb_{name}")
                            if scale is None:
                                nc.vector.tensor_tensor(
                                    xb, ps, brep[bnm][:, sl_], OP.add)
                            else:
                                tmp = sa.tile([P, 512], F32, tag="xbt",
                                              bufs=3, name=f"xbt_{name}")
                                nc.vector.tensor_scalar(tmp, ps, scale, None,
                                                        OP.mult)
                                nc.vector.tensor_tensor(
                                    xb, tmp, brep[bnm][:, sl_], OP.add)
                            src, scale = xb, None
                        kw = {} if scale is None else {"scale": scale}
                        E = sa.tile([P, 512], BF16, tag="E", bufs=4,
                                    name=f"E_{name}")
                        nc.scalar.activation(E, src, AF.Exp, **kw)
                        r = sa.tile([P, 512], BF16, tag="r", bufs=4,
                                    name=f"r_{name}")
                        nc.scalar.activation(r, src, AF.Relu, **kw)
                        tm = sa.tile([P, 512], BF16, tag="tm", bufs=4,
                                     name=f"t_{name}")
                        nc.vector.tensor_scalar(tm, E, 1.0, 1.0, OP.min,
                                                OP.subtract)
                        nc.vector.tensor_tensor(dst, r, tm, OP.add)

                    for half in range(2):
                        sl = slice(half * 512, (half + 1) * 512)

                        k_ps = pa.tile([P, 512], F32, tag="pj", bufs=3,
                                       name=f"k_ps{t}_{half}")
                        for c in range(NC):
                            nc.tensor.matmul(k_ps, xT[:, c, :],
                                             w_sb["wk"][:, c, sl],
                                             start=(c == 0), stop=(c == NC - 1))
                        elu_into(raw[:, sl], k_ps, None, f"k_{half}_{t}")

                        v_ps = pa.tile([P, 512], F32, tag="pj", bufs=3,
                                       name=f"v_ps{t}_{half}")
                        for c in range(NC):
                            nc.tensor.matmul(v_ps, xT[:, c, :],
                                             w_sb["wv"][:, c, sl],
                                             start=(c == 0), stop=(c == NC - 1))
                        if with_bias:
                            nc.vector.tensor_tensor(vflat[:, sl], v_ps,
                                                    brep["bv"][:, sl], OP.add)
                        else:
                            nc.scalar.copy(vflat[:, sl], v_ps)

                        q_ps = pa.tile([P, 512], F32, tag="pj", bufs=3,
                                       name=f"q_ps{t}_{half}")
                        for c in range(NC):
                            nc.tensor.matmul(q_ps, xT[:, c, :],
                                             w_sb["wq"][:, c, sl],
                                             start=(c == 0), stop=False)
                        nc.tensor.matmul(q_ps, nmrow, csq_sb[0:1, sl],
                                         start=False, stop=True)
                        elu_into(raw[:, 1024 + half * 512:1536 + half * 512],
                                 q_ps, rstd, f"q_{half}_{t}")

                    # l2 norms for k and q: rsqrt = exp(-0.5*ln(sumsq))
                    sq = sa.tile([P, 2 * HID], BF16, tag="sq", bufs=2,
                                 name=f"sq_{t}")
                    nc.vector.tensor_tensor(sq[:, 0:HID], raw[:, 0:HID],
                                            raw[:, 0:HID], OP.mult)
                    nc.vector.tensor_tensor(sq[:, HID:], raw[:, HID:],
                                            raw[:, HID:], OP.mult)
                    ss = sa.tile([P, 2 * NH], F32, tag="ss", bufs=3,
                                 name=f"ss_{t}")
                    sqv = sq[:].rearrange("p (h d) -> p h d", d=HD)
                    nc.vector.tensor_reduce(ss[:, 0:NH], sqv[:, 0:NH, :],
                                            mybir.AxisListType.X, OP.add)
                    nc.vector.tensor_reduce(ss[:, NH:], sqv[:, NH:, :],
                                            mybir.AxisListType.X, OP.add)
                    lnss = sa.tile([P, 2 * NH], F32, tag="lnss", bufs=3,
                                   name=f"lnss_{t}")
                    nc.scalar.activation(lnss, ss, AF.Ln)
                    rn = sa.tile([P, 2 * NH], BF16, tag="rn", bufs=3,
                                 name=f"rn_{t}")
                    nc.scalar.activation(rn, lnss, AF.Exp, scale=-0.5)

                    ek = sa.tile([P, NH, HD], BF16, tag="ek", bufs=2,
                                 name=f"ek_{t}")
                    nc.vector.tensor_tensor(
                        ek, raw[:, 0:HID].rearrange("p (h d) -> p h d", d=HD),
                        rn[:, 0:NH, None].to_broadcast((P, NH, HD)), OP.mult)
                    eq = sa.tile([P, NH, HD], BF16, tag="eq", bufs=2,
                                 name=f"eq_{t}")
                    nc.vector.tensor_tensor(
                        eq, raw[:, HID:].rearrange("p (h d) -> p h d", d=HD),
                        rn[:, NH:, None].to_broadcast((P, NH, HD)), OP.mult)
                    return ek, v_sb, eq

                def tile_tail(t, ek, v_sb, eq):
                    """kv-state pair matmuls + eq^T transposes for tile t."""
                    ekf = ek[:].rearrange("p h d -> p (h d)")
                    vf = v_sb[:].rearrange("p h d -> p (h d)")
                    for a in range(8):
                        nc.tensor.matmul(
                            kv_ps[:, a * P:(a + 1) * P],
                            ekf[:, a * P:(a + 1) * P],
                            vf[:, a * P:(a + 1) * P],
                            start=(t == 0 and a % 4 == 0), stop=(t == NT - 1),
                            skip_group_check=True)
                    eqf = eq[:].rearrange("p h d -> p (h d)")
                    for half in range(2):
                        tp2 = pa.tile([P, 4 * P], BF16, tag="tp", bufs=2,
                                      name=f"tpB_{t}_{half}")
                        for b_ in range(4):
                            blk = half * 4 + b_
                            nc.tensor.transpose(
                                tp2[:, b_ * P:(b_ + 1) * P],
                                eqf[:, blk * P:(blk + 1) * P], ident)
                        nc.vector.tensor_copy(
                            eqT[:, half * 4:(half + 1) * 4,
                                t * P:(t + 1) * P], tp2)

                prev = None
                for t in range(NT):
                    cur = tile_front(t)
                    if prev is not None:
                        tile_tail(prev[0], *prev[1])
                    prev = (t, cur)
                tile_tail(prev[0], *prev[1])

                # kv state -> SBUF bf16: diagonal blocks of each pair.
                # head 2a   -> kv_sb[0:64,   a*64:(a+1)*64]
                # head 2a+1 -> kv_sb[64:128, a*64:(a+1)*64]
                kvv = kv_ps[:].rearrange("p (a s) -> p a s", s=P)
                kvb = kv_sb[:].rearrange("p (a d) -> p a d", d=HD)
                nc.vector.tensor_copy(kvb[0:HD], kvv[0:HD, :, 0:HD])
                nc.vector.tensor_copy(kvb[HD:P], kvv[HD:P, :, HD:P])

            # ---------------- pass B ----------------
            with tc.tile_pool(name="sbufB", bufs=1) as sbp, \
                 tc.tile_pool(name="psumB", bufs=1, space="PSUM") as pb:
                for ch in range(NCHUNKS):
                    s0 = ch * CHUNK * P
                    ctxT = sbp.tile([P, NC, CHUNK * P], BF16, tag="ctx",
                                    bufs=2, name=f"ctxT{ch}")
                    for jt in range(NC):
                        c_pse = pb.tile([HD, CHUNK * P], F32, tag="ce",
                                        bufs=2, name=f"c_pse{ch}_{jt}")
                        c_pso = pb.tile([HD, CHUNK * P], F32, tag="co",
                                        bufs=2, name=f"c_pso{ch}_{jt}")
                        nc.tensor.matmul(
                            c_pse, kv_sb[0:HD, jt * HD:(jt + 1) * HD],
                            eqT[0:HD, jt, s0:s0 + CHUNK * P],
                            start=True, stop=True)
                        nc.tensor.matmul(
                            c_pso, kv_sb[HD:P, jt * HD:(jt + 1) * HD],
                            eqT[HD:P, jt, s0:s0 + CHUNK * P],
                            start=True, stop=True)
                        nc.scalar.copy(ctxT[0:HD, jt, :], c_pse)
                        nc.scalar.copy(ctxT[HD:P, jt, :], c_pso)

                    for tl in range(CHUNK):
                        t = ch * CHUNK + tl
                        xr = sbp.tile([P, HID], BF16, tag="xr", bufs=4,
                                      name=f"xr_{t}")
                        nc.sync.dma_start(xr, x_d.ap()[t * P:(t + 1) * P, :])
                        res = xr
                        if with_bias:
                            xb2 = sbp.tile([P, HID], BF16, tag="xb2", bufs=2,
                                           name=f"xb2_{t}")
                            nc.gpsimd.tensor_tensor(xb2, xr, brep["bo"],
                                                    OP.add)
                            res = xb2
                        outt = sbp.tile([P, HID], BF16, tag="osb", bufs=3,
                                        name=f"out_{t}")
                        for half in range(2):
                            sl = slice(half * 512, (half + 1) * 512)
                            o_ps = pb.tile([P, 512], F32, tag="po", bufs=3,
                                           name=f"o_ps{t}_{half}")
                            for c in range(NC):
                                nc.tensor.matmul(
                                    o_ps, ctxT[:, c, tl * P:(tl + 1) * P],
                                    w_sb["wo"][:, c, sl],
                                    start=(c == 0), stop=(c == NC - 1))
                            nc.vector.tensor_tensor(outt[:, sl], o_ps,
                                                    res[:, sl], OP.add)
                        nc.gpsimd.dma_start(
                            out_d.ap()[t * P:(t + 1) * P, :], outt)

    nc.compile()
    return nc


_RUNNER = {}


def _get_runner(loop_n=1, with_bias=False):
    key = (loop_n, with_bias)
    if key in _RUNNER:
        return _RUNNER[key]

    import jax
    from jax.sharding import Mesh, PartitionSpec
    from jax.experimental.shard_map import shard_map
    from concourse.bass2jax import _bass_exec_p, install_neuronx_cc_hook

    install_neuronx_cc_hook()
    nc = build_nc(loop_n=loop_n, with_bias=with_bias)

    in_names = []
    out_names = []
    out_avals = []
    for alloc in nc.m.functions[0].allocations:
        if not isinstance(alloc, mybir.MemoryLocationSet):
            continue
        name = alloc.memorylocations[0].name
        if alloc.kind == "ExternalInput":
            in_names.append(name)
        elif alloc.kind == "ExternalOutput":
            out_names.append(name)
            out_avals.append(
                jax.core.ShapedArray(tuple(alloc.tensor_shape),
                                     mybir.dt.np(alloc.dtype)))
    n_params = len(in_names)
    all_in_names = in_names + out_names

    def _body(*args):
        outs = _bass_exec_p.bind(
            *args,
            out_avals=tuple(out_avals),
            in_names=tuple(all_in_names),
            out_names=tuple(out_names),
            lowering_input_output_aliases=(),
            sim_require_finite=True,
            sim_require_nnan=True,
            nc=nc,
        )
        return tuple(outs)

    devices = jax.devices()[:B]
    mesh = Mesh(np.asarray(devices), ("core",))
    n_outs = len(out_names)
    fn = jax.jit(
        shard_map(
            _body, mesh=mesh,
            in_specs=(PartitionSpec("core"),) * (n_params + n_outs),
            out_specs=(PartitionSpec("core"),) * n_outs,
            check_rep=False,
        ),
        keep_unused=True,
    )
    _RUNNER[key] = (fn, in_names, out_names, out_avals)
    return _RUNNER[key]


def prep_inputs(input_tensor, attention_mask, ln_gamma, ln_beta,
                Wq, bq, Wk, bk, Wv, bv, Wo, bo):
    """Host-side static prep: transpose weights, fold gamma/beta/scale,
    convert to bf16."""
    import ml_dtypes
    bf = ml_dtypes.bfloat16
    f = np.float32
    x = np.asarray(input_tensor, f).astype(bf)
    g = np.asarray(ln_gamma, f)
    be = np.asarray(ln_beta, f)
    Wq = np.asarray(Wq, f); Wk = np.asarray(Wk, f)
    Wv = np.asarray(Wv, f); Wo = np.asarray(Wo, f)
    wqt = np.ascontiguousarray((Wq * g[None, :]).T).astype(bf)     # [i, j]
    wkt = np.ascontiguousarray(Wk.T).astype(bf)
    wvt = np.ascontiguousarray(Wv.T).astype(bf)
    wot = np.ascontiguousarray(
        Wo.T * np.float32(1.0 / np.sqrt(HD))).astype(bf)
    csq = wqt.astype(np.float64).sum(axis=0, keepdims=True).astype(bf)
    bq_eff = (np.asarray(bq, f) + be @ Wq.T).astype(f)
    bk = np.asarray(bk, f); bv = np.asarray(bv, f); bo = np.asarray(bo, f)
    with_bias = bool(np.any(bq_eff) or np.any(bk) or np.any(bv)
                     or np.any(bo))
    per_core = {
        "wqt": wqt, "wkt": wkt, "wvt": wvt, "wot": wot, "csq": csq,
    }
    if with_bias:
        per_core.update({
            "bq": bq_eff.reshape(1, HID), "bk": bk.reshape(1, HID),
            "bv": bv.reshape(1, HID), "bo": bo.reshape(1, HID),
        })
    return x, per_core, with_bias


def kernel(**inputs) -> np.ndarray:
    x, per_core, with_bias = prep_inputs(**inputs)
    fn, in_names, out_names, out_avals = _get_runner(with_bias=with_bias)

    concat_in = []
    for name in in_names:
        if name == "x":
            concat_in.append(x.reshape(B * S, HID))
        else:
            concat_in.append(np.concatenate([per_core[name]] * B, axis=0))
    concat_zeros = [
        np.zeros((B * av.shape[0], *av.shape[1:]), av.dtype) for av in out_avals
    ]
    out_arrs = fn(*concat_in, *concat_zeros)
    out = np.asarray(out_arrs[out_names.index("out")]).astype(np.float32)
    return out.reshape(B, S, HID)


# revision 9
# speedup vs baseline: 1.2586x; 1.0827x over previous
"""Trainium2 Bass kernel for nn_MultiHeadAttention_84791244358011.

Linear (ELU feature-map) attention:
    x_norm = LayerNorm(x)                      # eps=1e-12
    q = x_norm @ Wq.T + bq ; k,v = x @ W.T + b # per-head [S, 64]
    eq/ek = l2norm(elu(q/k)) per token over head_dim
    kv = ek^T @ v per head [64, 64]; ctx = eq @ kv / 8
    out = ctx @ Wo.T + bo + x

Sharding: data-parallel over batch B=8 — one batch element per NeuronCore,
no collectives.

v2 design (single pass, bf16 dataflow):
  - x converted to bf16 host-side (halves DMA; LN stats in fp32).
  - Weights pre-transposed + bf16 host-side:
        wqt[i,j] = Wq[j,i]*gamma[i]; wkt/wvt = W.T; wot = Wo.T/sqrt(64)
    every matmul contracts over the SBUF partition dim at 1 cycle/row.
  - LayerNorm folded into the q projection:
        q = rstd * (x @ wqt - mu * colsum(wqt))
    the -mu*colsum term is a rank-1 (K=1) matmul into the same PSUM
    accumulation; rstd rides the ACT `scale=` operand of the elu reads.
  - Single pass A per 128-token tile: transpose x; k/v/q projections;
    elu = Relu(ps) + (min(Exp(ps),1)-1); batched l2-norms with
    rsqrt = Exp(-0.5*Ln(ss)) on ACT (the act table pass is pinned to one
    table containing exp/ln/square/relu/copy — no table thrash);
    per-head-pair kv-state matmuls (8 of [128,128], diagonal blocks used);
    eq^T kept resident in SBUF (bf16) — no DRAM spill.
    PE work of tile t-1's tail (kv matmuls + eq^T transposes) is emitted
    after tile t's projections so the elu/norm chain of t-1 overlaps the
    PE-heavy front of t.
  - Pass B per 512-token chunk: ctx^T = kv @ eq^T; out = ctx^T.T @ wot + x.

Bias handling: when bq_eff (= bq + beta @ Wq.T), bk, bv, bo are all zero
(true for this problem's inputs) the bias adds are compiled out; a general
variant with the adds is built if any bias is nonzero.
"""

import functools

import numpy as np

import concourse.bass as bass
import concourse.mybir as mybir
import concourse.tile as tile
from concourse import bacc
from concourse.masks import make_identity

B, S, HID = 8, 4096, 1024
NH, HD = 16, 64
P = 128
NT = S // P            # 32 token tiles
NC = HID // P          # 8 feature chunks
CHUNK = 4              # token tiles per ctx chunk (512 tokens)
NCHUNKS = NT // CHUNK
LN_EPS = 1e-12

F32 = mybir.dt.float32
BF16 = mybir.dt.bfloat16
AF = mybir.ActivationFunctionType
OP = mybir.AluOpType

_ACT_PATCHED = False


def _patch_act_tables():
    """Pin the ACT table pass to one function set containing every func we
    use (exp/ln/square/relu/copy/identity), so it is loaded once instead of
    thrashing between the exp and ln sets. Set ids and contents are
    unchanged — other sets merely stop advertising our funcs."""
    global _ACT_PATCHED
    if _ACT_PATCHED:
        return
    import concourse.hw_specs as hws

    need = {AF.Exp, AF.Ln, AF.Square, AF.Relu, AF.Copy, AF.Identity}
    orig = hws.get_activation_tables

    @functools.cache
    def patched(arch):
        d = orig(arch)
        best = None
        for name, s in d.items():
            if need <= s:
                best = name
                break
        if best is None:
            return d
        return {name: (s if name == best else (s - need))
                for name, s in d.items()}

    bacc.get_activation_tables = patched
    hws.get_activation_tables = patched
    _ACT_PATCHED = True


def build_nc(loop_n=1, with_bias=False):
    _patch_act_tables()
    nc = bacc.Bacc("TRN2", target_bir_lowering=False, enable_partition_id=False)

    x_d = nc.dram_tensor("x", [S, HID], BF16, kind="ExternalInput")
    wqt_d = nc.dram_tensor("wqt", [HID, HID], BF16, kind="ExternalInput")
    wkt_d = nc.dram_tensor("wkt", [HID, HID], BF16, kind="ExternalInput")
    wvt_d = nc.dram_tensor("wvt", [HID, HID], BF16, kind="ExternalInput")
    wot_d = nc.dram_tensor("wot", [HID, HID], BF16, kind="ExternalInput")
    csq_d = nc.dram_tensor("csq", [1, HID], BF16, kind="ExternalInput")
    b_d = {}
    if with_bias:
        for nm in ("bq", "bk", "bv", "bo"):
            b_d[nm] = nc.dram_tensor(nm, [1, HID], F32, kind="ExternalInput")
    out_d = nc.dram_tensor("out", [S, HID], BF16, kind="ExternalOutput")

    import contextlib

    with tile.TileContext(nc) as tc, contextlib.ExitStack() as ctx:
        persist = ctx.enter_context(tc.tile_pool(name="persist", bufs=1))

        ident = persist.tile([P, P], BF16)
        make_identity(nc, ident)
        eqT = persist.tile([P, NC, S], BF16, name="eqT")      # 64KB/part
        kv_sb = persist.tile([P, (NH // 2) * HD], BF16, name="kv_sb")
        csq_sb = persist.tile([1, HID], BF16, name="csq_sb")
        nc.sync.dma_start(csq_sb, csq_d.ap())
        w_sb = {}
        for nm, d in (("wq", wqt_d), ("wk", wkt_d), ("wv", wvt_d),
                      ("wo", wot_d)):
            t_ = persist.tile([P, NC, HID], BF16, name=f"{nm}_sb")
            nc.sync.dma_start(t_, d.ap().rearrange("(c p) j -> p c j", p=P))
            w_sb[nm] = t_
        brep = {}
        if with_bias:
            for nm, d in b_d.items():
                t_ = persist.tile([P, HID], F32, name=f"{nm}_rep")
                h = d.ap()
                nc.gpsimd.dma_start(
                    t_, bass.AP(tensor=h.tensor, offset=h.offset,
                                ap=[[0, P], [1, HID]]))
                brep[nm] = t_

        _loop = tc.For_i(0, loop_n, 1) if loop_n > 1 else contextlib.nullcontext(0)
        with _loop:
            # ---------------- pass A ----------------
            with tc.tile_pool(name="sbufA", bufs=1) as sa, \
                 tc.tile_pool(name="psumA", bufs=1, space="PSUM") as pa:
                # kv state: head pairs a=0..7, [128, 128] block each; the
                # diagonal 64x64 blocks are the per-head kv states.
                kv_ps = pa.tile([P, 8 * P], F32, tag="kv", name="kv_ps")

                # x^T via grouped DMA-transpose straight from DRAM:
                # per CHUNK-tile group, per 128-col block c:
                #   [CHUNK*128 rows, 128 cols] -> [128, CHUNK*128]
                xT_g = {}

                def load_xT_group(g):
                    xTg = sa.tile([P, NC, CHUNK * P], BF16, tag="xTg",
                                  bufs=2, name=f"xTg_{g}")
                    r0 = g * CHUNK * P
                    for c in range(NC):
                        nc.sync.dma_start_transpose(
                            xTg[:, c, :],
                            x_d.ap()[r0:r0 + CHUNK * P,
                                     c * P:(c + 1) * P])
                    xT_g[g] = xTg

                def tile_front(t, eqc):
                    """DMA + stats + projections + elu + norms.
                    Returns (ek, v_sb) bf16 tiles for the tail."""
                    xt = sa.tile([P, HID], BF16, tag="x", bufs=3,
                                 name=f"x_{t}")
                    nc.scalar.dma_start(xt, x_d.ap()[t * P:(t + 1) * P, :])

                    tl_ = t % CHUNK
                    xTg = xT_g[t // CHUNK]
                    xT = xTg[:, :, tl_ * P:(tl_ + 1) * P]

                    # LayerNorm stats (fp32)
                    stats = sa.tile([P, 2, 6], F32, tag="st", bufs=4,
                                    name=f"st_{t}")
                    xg = xt[:].rearrange("p (g d) -> p g d", g=2)
                    for g in range(2):
                        nc.vector.bn_stats(stats[:, g, :], xg[:, g, :])
                    mv = sa.tile([P, 2], F32, tag="mv", bufs=4, name=f"mv_{t}")
                    nc.vector.bn_aggr(mv, stats)
                    vpe = sa.tile([P, 1], F32, tag="vpe", bufs=4,
                                  name=f"vpe_{t}")
                    nc.vector.tensor_scalar(vpe, mv[:, 1:2], LN_EPS, None,
                                            OP.add)
                    lnv = sa.tile([P, 1], F32, tag="lnv", bufs=4,
                                  name=f"lnv_{t}")
                    nc.scalar.activation(lnv, vpe, AF.Ln)
                    rstd = sa.tile([P, 1], F32, tag="rstd", bufs=4,
                                   name=f"rstd_{t}")
                    nc.scalar.activation(rstd, lnv, AF.Exp, scale=-0.5)
                    negmu = sa.tile([P, 1], BF16, tag="nmu", bufs=4,
                                    name=f"nmu_{t}")
                    nc.vector.tensor_scalar(negmu, mv[:, 0:1], -1.0, None,
                                            OP.mult)
                    tpn = pa.tile([P, P], BF16, tag="tpn", bufs=2,
                                  name=f"tpn_{t}")
                    nc.tensor.transpose(tpn[0:1, 0:P], negmu, ident)
                    nmrow = sa.tile([1, P], BF16, tag="nmrow", bufs=3,
                                    name=f"nmrow_{t}")
                    nc.vector.tensor_copy(nmrow, tpn[0:1, 0:P])

                    # raw = [elu(k) | elu(q)] packed [P, 2048]
                    raw = sa.tile([P, 2 * HID], BF16, tag="raw", bufs=2,
                                  name=f"raw_{t}")
                    v_sb = sa.tile([P, NH, HD], BF16, tag="vsb", bufs=2,
                                   name=f"v_{t}")
                    vflat = v_sb[:].rearrange("p h d -> p (h d)")

                    def elu_into(dst, ps, scale, name):
                        # dst = Relu(ps*scale) + (min(Exp(ps*scale),1) - 1)
                        src = ps
                        if with_bias:
                            # general path: materialize ps*scale + bias first
                            bnm = "bq" if name.startswith("q") else "bk"
                            sl_ = slice(int(name.split("_")[1]) * 512,
                                        (int(name.split("_")[1]) + 1) * 512)
                            xb = sa.tile([P, 512], BF16, tag="xb", bufs=3,
                                         name=f"xb_{name}")
                            if scale is None:
                                nc.vector.tensor_tensor(
                                    xb, ps, brep[bnm][:, sl_], OP.add)
                            else:
                                tmp = sa.tile([P, 512], F32, tag="xbt",
                                              bufs=3, name=f"xbt_{name}")
                                nc.vector.tensor_scalar(tmp, ps, scale, None,
                                                        OP.mult)
                                nc.vector.tensor_tensor(
                                    xb, tmp, brep[bnm][:, sl_], OP.add)
                            src, scale = xb, None
                        kw = {} if scale is None else {"scale": scale}
                        E = sa.tile([P, 512], BF16, tag="E", bufs=3,
                                    name=f"E_{name}")
                        nc.scalar.activation(E, src, AF.Exp, **kw)
                        r = sa.tile([P, 512], BF16, tag="r", bufs=3,
                                    name=f"r_{name}")
                        nc.scalar.activation(r, src, AF.Relu, **kw)
                        tm = sa.tile([P, 512], BF16, tag="tm", bufs=3,
                                     name=f"t_{name}")
                        nc.vector.tensor_scalar(tm, E, 1.0, 1.0, OP.min,
                                                OP.subtract)
                        nc.vector.tensor_tensor(dst, r, tm, OP.add)

                    for half in range(2):
                        sl = slice(half * 512, (half + 1) * 512)

                        k_ps = pa.tile([P, 512], F32, tag="pj", bufs=4,
                                       name=f"k_ps{t}_{half}")
                        for c in range(NC):
                            nc.tensor.matmul(k_ps, xT[:, c, :],
                                             w_sb["wk"][:, c, sl],
                                             start=(c == 0), stop=(c == NC - 1))
                        elu_into(raw[:, sl], k_ps, None, f"k_{half}_{t}")

                        v_ps = pa.tile([P, 512], F32, tag="pj", bufs=4,
                                       name=f"v_ps{t}_{half}")
                        for c in range(NC):
                            nc.tensor.matmul(v_ps, xT[:, c, :],
                                             w_sb["wv"][:, c, sl],
                                             start=(c == 0), stop=(c == NC - 1))
                        if with_bias:
                            nc.vector.tensor_tensor(vflat[:, sl], v_ps,
                                                    brep["bv"][:, sl], OP.add)
                        else:
                            nc.scalar.copy(vflat[:, sl], v_ps)

                        q_ps = pa.tile([P, 512], F32, tag="pj", bufs=4,
                                       name=f"q_ps{t}_{half}")
                        for c in range(NC):
                            nc.tensor.matmul(q_ps, xT[:, c, :],
                                             w_sb["wq"][:, c, sl],
                                             start=(c == 0), stop=False)
                        nc.tensor.matmul(q_ps, nmrow, csq_sb[0:1, sl],
                                         start=False, stop=True)
                        elu_into(raw[:, 1024 + half * 512:1536 + half * 512],
                                 q_ps, rstd, f"q_{half}_{t}")

                    # l2 norms for k and q: rsqrt = exp(-0.5*ln(sumsq))
                    sq = sa.tile([P, 2 * HID], BF16, tag="sq", bufs=2,
                                 name=f"sq_{t}")
                    nc.vector.tensor_tensor(sq[:, 0:HID], raw[:, 0:HID],
                                            raw[:, 0:HID], OP.mult)
                    nc.vector.tensor_tensor(sq[:, HID:], raw[:, HID:],
                                            raw[:, HID:], OP.mult)
                    ss = sa.tile([P, 2 * NH], F32, tag="ss", bufs=3,
                                 name=f"ss_{t}")
                    sqv = sq[:].rearrange("p (h d) -> p h d", d=HD)
                    nc.vector.tensor_reduce(ss[:, 0:NH], sqv[:, 0:NH, :],
                                            mybir.AxisListType.X, OP.add)
                    nc.vector.tensor_reduce(ss[:, NH:], sqv[:, NH:, :],
                                            mybir.AxisListType.X, OP.add)
                    lnss = sa.tile([P, 2 * NH], F32, tag="lnss", bufs=3,
                                   name=f"lnss_{t}")
                    nc.scalar.activation(lnss, ss, AF.Ln)
                    rn = sa.tile([P, 2 * NH], BF16, tag="rn", bufs=3,
                                 name=f"rn_{t}")
                    nc.scalar.activation(rn, lnss, AF.Exp, scale=-0.5)

                    ek = sa.tile([P, NH, HD], BF16, tag="ek", bufs=2,
                                 name=f"ek_{t}")
                    nc.vector.tensor_tensor(
                        ek, raw[:, 0:HID].rearrange("p (h d) -> p h d", d=HD),
                        rn[:, 0:NH, None].to_broadcast((P, NH, HD)), OP.mult)
                    # eq written into the chunk staging tile (block-major
                    # cols tl*128+j) for the chunk-end DMA transpose.
                    eqv = eqc[:, :, tl_ * P:(tl_ + 1) * P].rearrange(
                        "p c (s d) -> p c s d", d=HD)
                    nc.vector.tensor_tensor(
                        eqv, raw[:, HID:].rearrange("p (c s d) -> p c s d",
                                                    s=2, d=HD),
                        rn[:, NH:].rearrange("p (c s) -> p c s", s=2)[
                            :, :, :, None].to_broadcast((P, NC, 2, HD)),
                        OP.mult)
                    return ek, v_sb

                def tile_tail(t, ek, v_sb):
                    """kv-state pair matmuls for tile t."""
                    ekf = ek[:].rearrange("p h d -> p (h d)")
                    vf = v_sb[:].rearrange("p h d -> p (h d)")
                    for a in range(8):
                        nc.tensor.matmul(
                            kv_ps[:, a * P:(a + 1) * P],
                            ekf[:, a * P:(a + 1) * P],
                            vf[:, a * P:(a + 1) * P],
                            start=(t == 0 and a % 4 == 0), stop=(t == NT - 1),
                            skip_group_check=True)

                load_xT_group(0)
                prev = None
                eqc = None
                for t in range(NT):
                    g, tl = t // CHUNK, t % CHUNK
                    if tl == 0:
                        if g + 1 < NCHUNKS:
                            load_xT_group(g + 1)
                        eqc = sa.tile([P, NC, CHUNK * P], BF16, tag="eqc",
                                      bufs=2, name=f"eqc_{g}")
                    cur = tile_front(t, eqc)
                    if prev is not None:
                        tile_tail(prev[0], *prev[1])
                    prev = (t, cur)
                    if tl == CHUNK - 1:
                        s0 = g * CHUNK * P
                        for c in range(NC):
                            nc.sync.dma_start_transpose(
                                eqT[:, c, s0:s0 + CHUNK * P].rearrange(
                                    "p (tl m) -> p tl m", m=P),
                                eqc[:, c, :])
                tile_tail(prev[0], *prev[1])

                # kv state -> SBUF bf16: diagonal blocks of each pair.
                # head 2a   -> kv_sb[0:64,   a*64:(a+1)*64]
                # head 2a+1 -> kv_sb[64:128, a*64:(a+1)*64]
                kvv = kv_ps[:].rearrange("p (a s) -> p a s", s=P)
                kvb = kv_sb[:].rearrange("p (a d) -> p a d", d=HD)
                nc.vector.tensor_copy(kvb[0:HD], kvv[0:HD, :, 0:HD])
                nc.vector.tensor_copy(kvb[HD:P], kvv[HD:P, :, HD:P])

            # ---------------- pass B ----------------
            with tc.tile_pool(name="sbufB", bufs=1) as sbp, \
                 tc.tile_pool(name="psumB", bufs=1, space="PSUM") as pb:
                for ch in range(NCHUNKS):
                    s0 = ch * CHUNK * P
                    ctxT = sbp.tile([P, NC, CHUNK * P], BF16, tag="ctx",
                                    bufs=2, name=f"ctxT{ch}")
                    for jt in range(NC):
                        c_pse = pb.tile([HD, CHUNK * P], F32, tag="ce",
                                        bufs=2, name=f"c_pse{ch}_{jt}")
                        c_pso = pb.tile([HD, CHUNK * P], F32, tag="co",
                                        bufs=2, name=f"c_pso{ch}_{jt}")
                        nc.tensor.matmul(
                            c_pse, kv_sb[0:HD, jt * HD:(jt + 1) * HD],
                            eqT[0:HD, jt, s0:s0 + CHUNK * P],
                            start=True, stop=True)
                        nc.tensor.matmul(
                            c_pso, kv_sb[HD:P, jt * HD:(jt + 1) * HD],
                            eqT[HD:P, jt, s0:s0 + CHUNK * P],
                            start=True, stop=True)
                        nc.scalar.copy(ctxT[0:HD, jt, :], c_pse)
                        nc.scalar.copy(ctxT[HD:P, jt, :], c_pso)

                    for tl in range(CHUNK):
                        t = ch * CHUNK + tl
                        xr = sbp.tile([P, HID], BF16, tag="xr", bufs=4,
                                      name=f"xr_{t}")
                        nc.sync.dma_start(xr, x_d.ap()[t * P:(t + 1) * P, :])
                        res = xr
                        if with_bias:
                            xb2 = sbp.tile([P, HID], BF16, tag="xb2", bufs=2,
                                           name=f"xb2_{t}")
                            nc.gpsimd.tensor_tensor(xb2, xr, brep["bo"],
                                                    OP.add)
                            res = xb2
                        outt = sbp.tile([P, HID], BF16, tag="osb", bufs=3,
                                        name=f"out_{t}")
                        for half in range(2):
                            sl = slice(half * 512, (half + 1) * 512)
                            o_ps = pb.tile([P, 512], F32, tag="po", bufs=3,
                                           name=f"o_ps{t}_{half}")
                            for c in range(NC):
                                nc.tensor.matmul(
                                    o_ps, ctxT[:, c, tl * P:(tl + 1) * P],
                                    w_sb["wo"][:, c, sl],
                                    start=(c == 0), stop=(c == NC - 1))
                            nc.vector.tensor_tensor(outt[:, sl], o_ps,
                                                    res[:, sl], OP.add)
                        nc.gpsimd.dma_start(
                            out_d.ap()[t * P:(t + 1) * P, :], outt)

    nc.compile()
    return nc


_RUNNER = {}


def _get_runner(loop_n=1, with_bias=False):
    key = (loop_n, with_bias)
    if key in _RUNNER:
        return _RUNNER[key]

    import jax
    from jax.sharding import Mesh, PartitionSpec
    from jax.experimental.shard_map import shard_map
    from concourse.bass2jax import _bass_exec_p, install_neuronx_cc_hook

    install_neuronx_cc_hook()
    nc = build_nc(loop_n=loop_n, with_bias=with_bias)

    in_names = []
    out_names = []
    out_avals = []
    for alloc in nc.m.functions[0].allocations:
        if not isinstance(alloc, mybir.MemoryLocationSet):
            continue
        name = alloc.memorylocations[0].name
        if alloc.kind == "ExternalInput":
            in_names.append(name)
        elif alloc.kind == "ExternalOutput":
            out_names.append(name)
            out_avals.append(
                jax.core.ShapedArray(tuple(alloc.tensor_shape),
                                     mybir.dt.np(alloc.dtype)))
    n_params = len(in_names)
    all_in_names = in_names + out_names

    def _body(*args):
        outs = _bass_exec_p.bind(
            *args,
            out_avals=tuple(out_avals),
            in_names=tuple(all_in_names),
            out_names=tuple(out_names),
            lowering_input_output_aliases=(),
            sim_require_finite=True,
            sim_require_nnan=True,
            nc=nc,
        )
        return tuple(outs)

    devices = jax.devices()[:B]
    mesh = Mesh(np.asarray(devices), ("core",))
    n_outs = len(out_names)
    fn = jax.jit(
        shard_map(
            _body, mesh=mesh,
            in_specs=(PartitionSpec("core"),) * (n_params + n_outs),
            out_specs=(PartitionSpec("core"),) * n_outs,
            check_rep=False,
        ),
        keep_unused=True,
    )
    _RUNNER[key] = (fn, in_names, out_names, out_avals)
    return _RUNNER[key]


def prep_inputs(input_tensor, attention_mask, ln_gamma, ln_beta,
                Wq, bq, Wk, bk, Wv, bv, Wo, bo):
    """Host-side static prep: transpose weights, fold gamma/beta/scale,
    convert to bf16."""
    import ml_dtypes
    bf = ml_dtypes.bfloat16
    f = np.float32
    x = np.asarray(input_tensor, f).astype(bf)
    g = np.asarray(ln_gamma, f)
    be = np.asarray(ln_beta, f)
    Wq = np.asarray(Wq, f); Wk = np.asarray(Wk, f)
    Wv = np.asarray(Wv, f); Wo = np.asarray(Wo, f)
    wqt = np.ascontiguousarray((Wq * g[None, :]).T).astype(bf)     # [i, j]
    wkt = np.ascontiguousarray(Wk.T).astype(bf)
    wvt = np.ascontiguousarray(Wv.T).astype(bf)
    wot = np.ascontiguousarray(
        Wo.T * np.float32(1.0 / np.sqrt(HD))).astype(bf)
    csq = wqt.astype(np.float64).sum(axis=0, keepdims=True).astype(bf)
    bq_eff = (np.asarray(bq, f) + be @ Wq.T).astype(f)
    bk = np.asarray(bk, f); bv = np.asarray(bv, f); bo = np.asarray(bo, f)
    with_bias = bool(np.any(bq_eff) or np.any(bk) or np.any(bv)
                     or np.any(bo))
    per_core = {
        "wqt": wqt, "wkt": wkt, "wvt": wvt, "wot": wot, "csq": csq,
    }
    if with_bias:
        per_core.update({
            "bq": bq_eff.reshape(1, HID), "bk": bk.reshape(1, HID),
            "bv": bv.reshape(1, HID), "bo": bo.reshape(1, HID),
        })
    return x, per_core, with_bias


def kernel(**inputs) -> np.ndarray:
    x, per_core, with_bias = prep_inputs(**inputs)
    fn, in_names, out_names, out_avals = _get_runner(with_bias=with_bias)

    concat_in = []
    for name in in_names:
        if name == "x":
            concat_in.append(x.reshape(B * S, HID))
        else:
            concat_in.append(np.concatenate([per_core[name]] * B, axis=0))
    concat_zeros = [
        np.zeros((B * av.shape[0], *av.shape[1:]), av.dtype) for av in out_avals
    ]
    out_arrs = fn(*concat_in, *concat_zeros)
    out = np.asarray(out_arrs[out_names.index("out")]).astype(np.float32)
    return out.reshape(B, S, HID)
